# revision 24
# baseline (speedup 1.0000x reference)
"""MinGRU cell on 8 Trainium2 NeuronCores (Bass/Tile).

Math (per batch b, hidden h):
    gz = x @ W_z^T ; gh = x @ W_h^T                 (two GEMMs, K=D=1024)
    z  = sigmoid(gz + b_z)
    h_t = (1 - z_t) * h_{t-1} + z_t * (gh_t + b_h)  (affine scan over T)

Distribution: data-parallel over batch B=16 -> 2 batches per core, weights
replicated; no cross-core communication.

Per-core pipeline, software-pipelined over a step table (7 steps of 512
tokens + 2 final steps of 256 so the output drain of the last tokens
overlaps compute; 256-col matmuls still hide their LDWEIGHTS):
  x [t,d] --SWDGE cast-load--> bf16 --PE transpose--> xT [d,t]
  GEMMs with W^T stationary (bf16, fp32 PSUM accumulation), output [h, t]
  ACT: a = sigmoid(-gz - b_z) = 1-z ; z = sigmoid(gz + b_z)
  DVE: bsc = (gh + b_h) * z ; h = tensor_tensor_scan(a, bsc) along t
  PE transpose h back to [t, h], copy to fp32 (ACT/DVE split), DMA out
  per h-half.

Steady state runs the PE at ~99% of the bf16 roofline (GEMM matmuls issue
every 216ns, transposes every 56ns, stationary loads hidden). Prologue:
x step-0 loads first (they gate the first GEMM), W^T built in bf16 from
per-128-row SWDGE chunk loads, W transposes emitted behind the step-0
GEMM pairs, junk-matmul warmup on a memset tile from ~4.7us so the HAM
clock gate opens once and stays open.
"""

import sys

sys.path.insert(0, "/opt/trn_rl_repo")

from contextlib import ExitStack

import numpy as np

import concourse.bass as bass
import concourse.mybir as mybir
import concourse.tile as tile
from concourse import bacc
from concourse.bass import ts, ds
from concourse.bass_utils import run_bass_kernel_spmd
from concourse.masks import make_identity

B, T, D, H = 16, 2048, 1024, 1024
NCORES = 8
B_LOC = B // NCORES  # 2
P = 128
TC = 512  # max tokens per step
TSUB = TC // P  # 4
DC = D // P  # 8 contraction chunks
HC = H // P  # 8 hidden chunks
HH = H // 2
NWARM = 75  # junk matmuls to open the PE clock gate during the DMA prologue

# Step table: (batch, start token, length). The last 512 tokens run as two
# 256-token steps to halve the final output drain.
STEPS = (
    [(0, t0, TC) for t0 in range(0, T, TC)]
    + [(1, t0, TC) for t0 in range(0, T - TC, TC)]
    + [(1, T - TC, TC // 2), (1, T - TC // 2, TC // 2)]
)
NSTEP = len(STEPS)  # 9

F32 = mybir.dt.float32
BF16 = mybir.dt.bfloat16
AF = mybir.ActivationFunctionType
OP = mybir.AluOpType

_CACHE = {}


class _State:
    pass


def _mingru_tile(tc, out, x, h0, wz, bz, wh, bh):
    nc = tc.nc
    st = _State()

    with ExitStack() as ctx:
        consts = ctx.enter_context(tc.tile_pool(name="consts", bufs=1))

        # Junk-warmup operand: a memset tile is ready ~4us before the
        # iota-built identity, letting the PE clock ramp start earlier.
        junk_mv = consts.tile([P, P], BF16, name="junk_mv")
        nc.gpsimd.memset(junk_mv, 0)

        # --- SBUF pools --------------------------------------------------
        wn_p = ctx.enter_context(tc.tile_pool(name="wn", bufs=1))
        xn_p = ctx.enter_context(tc.tile_pool(name="xn", bufs=2))
        xt_p = ctx.enter_context(tc.tile_pool(name="xt", bufs=2))
        azb_p = ctx.enter_context(tc.tile_pool(name="azb", bufs=2))
        scan_p = ctx.enter_context(tc.tile_pool(name="scan", bufs=2))
        on_p = ctx.enter_context(tc.tile_pool(name="on", bufs=3))

        st.xn = {}  # step -> [j natural x tiles [P, D] bf16]
        st.xt = {}  # step -> [4 xT pair tiles [P, 2*tl] bf16] (dc pair p)
        st.scan = {}  # step -> [8 scan tiles [P, tl] bf16]
        st.wt = {"z": [None] * HC, "h": [None] * HC}
        st.wn = {"z": [None] * HC, "h": [None] * HC}

        def sect_A(s):  # x cast-loads (SWDGE)
            b, t0, tl = STEPS[s]
            tiles = []
            for j in range(tl // P):
                xt_nat = xn_p.tile([P, D], BF16, tag=f"xn{j}", name=f"xn_{s}_{j}")
                nc.gpsimd.dma_start(
                    out=xt_nat, in_=x[b, ds(t0 + j * P, P), :]
                )
                tiles.append(xt_nat)
            st.xn[s] = tiles

        def load_w_chunk(hc):  # HWDGE fp32 load of one [128h, D] chunk of each W
            # Plain fp32 on the sync queue: the SWDGE cast path runs at only
            # ~210 GB/s and starves the step-0/1 W^T builds.
            for w_ap, wi in ((wz, "z"), (wh, "h")):
                t_ = wn_p.tile([P, D], F32, tag=f"wn_{wi}_{hc}",
                               name=f"wn_{wi}_{hc}")
                nc.sync.dma_start(out=t_, in_=w_ap[ts(hc, P), :])
                st.wn[wi][hc] = t_

        def build_wt(wi, hc):  # PE-transpose one W chunk (fp32), cast to bf16
            wnt = st.wn[wi][hc]
            wt_sb = consts.tile([P, D], BF16, name=f"wt_{wi}_{hc}")
            for half in range(2):
                pw = pxt_p.tile([P, D // 2], F32, tag="pxt",
                                name=f"pw_{wi}_{hc}_{half}")
                for k in range(DC // 2):
                    dc = half * (DC // 2) + k
                    nc.tensor.transpose(
                        pw[:, ts(k, P)], wnt[:, ts(dc, P)], id_f32
                    )
                # W_z copies on ACT, W_h copies on DVE (keeps both queues
                # clear of cross-waits against the merged GEMM stream).
                dst = wt_sb[:, ds(half * (D // 2), D // 2)]
                if wi == "z":
                    nc.scalar.copy(dst, pw)
                else:
                    nc.vector.tensor_copy(dst, pw)
            st.wt[wi][hc] = wt_sb

        def sect_B(s):  # x transposes (PE) into [P, 2*tl] pair tiles
            b, t0, tl = STEPS[s]
            jn = tl // P
            xn = st.xn.pop(s)
            tiles = []
            for p_ in range(DC // 2):
                pxt = pxt_p.tile([P, H], BF16, tag="pxt", name=f"pxt_{s}_{p_}")
                for q in range(2):
                    for j in range(jn):
                        nc.tensor.transpose(
                            pxt[:, ds(q * tl + j * P, P)],
                            xn[j][:, ts(2 * p_ + q, P)],
                            id_bf,
                        )
                xt_sb = xt_p.tile([P, 2 * TC], BF16, tag=f"xt{p_}",
                                  name=f"xt_{s}_{p_}")
                # split psum->SBUF copies across ACT / DVE
                if p_ < 2:
                    nc.scalar.copy(xt_sb[:, : 2 * tl], pxt[:, : 2 * tl])
                else:
                    nc.vector.tensor_copy(xt_sb[:, : 2 * tl], pxt[:, : 2 * tl])
                tiles.append(xt_sb)
            st.xt[s] = tiles

        def gemm(s, hc, which):
            b, t0, tl = STEPS[s]
            xts = st.xt[s]
            wt = st.wt[which][hc]
            pool = pz_p if which == "z" else ph_p
            psum = pool.tile(
                [P, TC], F32, tag="pz" if which == "z" else "ph",
                name=f"ps{which}_{s}_{hc}",
            )
            for dc in range(DC):
                nc.tensor.matmul(
                    psum[:, :tl],
                    wt[:, ts(dc, P)],
                    xts[dc // 2][:, ds((dc % 2) * tl, tl)],
                    start=(dc == 0),
                    stop=(dc == DC - 1),
                )
            return psum

        def sect_post(s, hc, psum_z, psum_h):
            b, t0, tl = STEPS[s]
            a_sb = azb_p.tile([P, TC], F32, tag="a", name=f"a_{s}_{hc}")
            nc.scalar.activation(
                a_sb[:, :tl], psum_z[:, :tl], AF.Sigmoid,
                bias=nbz_sb[:, hc : hc + 1], scale=-1.0,
            )
            z_sb = azb_p.tile([P, TC], F32, tag="z", name=f"z_{s}_{hc}")
            nc.scalar.activation(
                z_sb[:, :tl], psum_z[:, :tl], AF.Sigmoid,
                bias=bz_sb[:, hc : hc + 1], scale=1.0,
            )
            bsc = azb_p.tile([P, TC], F32, tag="b", name=f"b_{s}_{hc}")
            nc.vector.scalar_tensor_tensor(
                bsc[:, :tl], psum_h[:, :tl], bh_sb[:, hc : hc + 1],
                z_sb[:, :tl], op0=OP.add, op1=OP.mult,
            )
            # bf16 scan output: the scan's accumulator state is fp32 in HW
            # regardless of out dtype, so only stored values round (~2^-9).
            sc = scan_p.tile([P, TC], BF16, tag=f"sc{hc}", name=f"sc_{s}_{hc}")
            if t0 == 0:
                init = hp_sb[:, b * HC + hc : b * HC + hc + 1]
            else:
                ptl = STEPS[s - 1][2]
                init = st.scan[s - 1][hc][:, ptl - 1 : ptl]
            nc.vector.tensor_tensor_scan(
                sc[:, :tl], a_sb[:, :tl], bsc[:, :tl], init,
                op0=OP.mult, op1=OP.add,
            )
            st.scan.setdefault(s, [None] * HC)[hc] = sc

        def sect_E(s):  # out transposes (PE), copy to f32, store per h-half
            b, t0, tl = STEPS[s]
            scans = st.scan[s]
            for j in range(tl // P):
                po = pxt_p.tile([P, H], BF16, tag="pxt", name=f"po_{s}_{j}")
                for hc in range(HC):
                    nc.tensor.transpose(
                        po[:, ts(hc, P)], scans[hc][:, ts(j, P)], id_bf
                    )
                for hh in range(2):
                    on = on_p.tile([P, HH], F32, tag=f"on{hh}",
                                   name=f"on_{s}_{j}_{hh}")
                    # split the psum->fp32 casts across ACT / DVE
                    if hh == 0:
                        nc.scalar.copy(on, po[:, ds(0, HH)])
                    else:
                        nc.vector.tensor_copy(on, po[:, ds(HH, HH)])
                    nc.sync.dma_start(
                        out=out[b, ds(t0 + j * P, P), ds(hh * HH, HH)],
                        in_=on,
                    )
            if s - 1 in st.scan:
                del st.scan[s - 1]

        # --- prologue ----------------------------------------------------
        # Pool (SWDGE) queue order = criticality: x step 0 first (it gates
        # the first GEMM via the x transposes), then identity (gpsimd iota,
        # before the W chunk generations), then the W chunks.
        sect_A(0)

        id_bf = consts.tile([P, P], BF16)
        make_identity(nc, id_bf)
        id_f32 = consts.tile([P, P], F32)
        make_identity(nc, id_f32)

        # W chunk 0 next (gates the first GEMM pair), then x step 1 --
        # otherwise it queues behind all 16 W chunk DMAs and step 1 can
        # stall on it at the step boundary.
        load_w_chunk(0)
        sect_A(1)
        for hc in range(1, HC):
            load_w_chunk(hc)

        bz_sb = consts.tile([P, HC], F32)
        nc.sync.dma_start(out=bz_sb, in_=bz.rearrange("(c p) -> p c", p=P))
        bh_sb = consts.tile([P, HC], F32)
        nc.sync.dma_start(out=bh_sb, in_=bh.rearrange("(c p) -> p c", p=P))
        nbz_sb = consts.tile([P, HC], F32)
        nc.vector.tensor_scalar_mul(nbz_sb, bz_sb, -1.0)
        hp_sb = consts.tile([P, B_LOC * HC], F32)
        nc.sync.dma_start(out=hp_sb, in_=h0.rearrange("b (c p) -> p (b c)", p=P))

        # HAM warmup: back-to-back junk matmuls so the PE clock gate opens
        # while the prologue DMAs stream. The warm PSUM pool must close
        # before pz/ph/pxt open (8-bank budget).
        with tc.tile_pool(name="warm", bufs=1, space="PSUM") as warm_p, \
             tc.tile_pool(name="wdram", bufs=1, space="DRAM") as wdram_p:
            junk_ps = warm_p.tile([P, P], F32, name="junk_ps")
            for i in range(NWARM):
                nc.tensor.matmul(
                    junk_ps, junk_mv, junk_mv, start=(i == 0), stop=(i == NWARM - 1)
                )
            junk_sb = consts.tile([P, P], F32, name="junk_sb")
            nc.vector.tensor_copy(junk_sb, junk_ps)
            junk_dr = wdram_p.tile([P, P], F32, name="junk_dr")
            nc.sync.dma_start(out=junk_dr, in_=junk_sb)

        # PSUM: pz(3) + ph(3) + pxt(2) = 8 banks
        pz_p = ctx.enter_context(tc.tile_pool(name="pz", bufs=3, space="PSUM"))
        ph_p = ctx.enter_context(tc.tile_pool(name="ph", bufs=3, space="PSUM"))
        pxt_p = ctx.enter_context(tc.tile_pool(name="pxt", bufs=2, space="PSUM"))

        sect_B(0)
        # W^T chunks 0-1 ahead of the GEMM stream (2-chunk lookahead; the
        # remaining chunks are emitted behind the merged-step GEMM pairs).
        for hc in range(2):
            build_wt("z", hc)
            build_wt("h", hc)
        sect_B(1)

        # --- steps 0+1, merged hc-major ----------------------------------
        # The SWDGE cast W stream delivers one (W_z, W_h) chunk pair per
        # ~4.8us, while one step consumes a pair every ~3.4us of GEMM. By
        # running steps 0 and 1 hc-major (both steps' GEMMs per chunk), each
        # arrival feeds ~6.8us of PE work and the PE never starves on W.
        for hc in range(HC):
            if hc == 0:
                sect_A(2)
            if hc == 4:
                sect_A(3)
            for s01 in (0, 1):
                psum_z = gemm(s01, hc, "z")
                psum_h = gemm(s01, hc, "h")
                sect_post(s01, hc, psum_z, psum_h)
            if hc + 2 < HC:
                build_wt("z", hc + 2)
                build_wt("h", hc + 2)
            if hc == 6:
                sect_B(2)
        sect_E(0)

        # --- steady state ------------------------------------------------
        for s in range(2, NSTEP):
            if s + 2 < NSTEP:
                sect_A(s + 2)
            for hc in range(HC):
                psum_z = gemm(s, hc, "z")
                psum_h = gemm(s, hc, "h")
                sect_post(s, hc, psum_z, psum_h)
                if hc == 2:
                    sect_E(s - 1)
                if s + 1 < NSTEP and hc == 4:
                    sect_B(s + 1)
        sect_E(NSTEP - 1)


def build():
    if "nc" in _CACHE:
        return _CACHE["nc"]
    nc = bacc.Bacc(
        "TRN2", target_bir_lowering=False, debug=False, num_devices=NCORES
    )
    x = nc.dram_tensor("x", [B_LOC, T, D], F32, kind="ExternalInput").ap()
    h0 = nc.dram_tensor("h0", [B_LOC, H], F32, kind="ExternalInput").ap()
    wz = nc.dram_tensor("wz", [H, D], F32, kind="ExternalInput").ap()
    bz = nc.dram_tensor("bz", [H], F32, kind="ExternalInput").ap()
    wh = nc.dram_tensor("wh", [H, D], F32, kind="ExternalInput").ap()
    bh = nc.dram_tensor("bh", [H], F32, kind="ExternalInput").ap()
    out = nc.dram_tensor("out", [B_LOC, T, H], F32, kind="ExternalOutput").ap()
    with tile.TileContext(nc) as tctx:
        _mingru_tile(tctx, out, x, h0, wz, bz, wh, bh)
    nc.compile()
    _CACHE["nc"] = nc
    return nc


def make_in_maps(x, h_prev, W_z, b_z, W_h, b_h):
    x = np.ascontiguousarray(np.asarray(x, dtype=np.float32))
    h_prev = np.ascontiguousarray(np.asarray(h_prev, dtype=np.float32))
    W_z = np.ascontiguousarray(np.asarray(W_z, dtype=np.float32))
    b_z = np.ascontiguousarray(np.asarray(b_z, dtype=np.float32))
    W_h = np.ascontiguousarray(np.asarray(W_h, dtype=np.float32))
    b_h = np.ascontiguousarray(np.asarray(b_h, dtype=np.float32))
    in_maps = []
    for c in range(NCORES):
        sl = slice(c * B_LOC, (c + 1) * B_LOC)
        in_maps.append(
            {
                "x": x[sl],
                "h0": h_prev[sl],
                "wz": W_z,
                "bz": b_z,
                "wh": W_h,
                "bh": b_h,
            }
        )
    return in_maps


def kernel(x, h_prev, W_z, b_z, W_h, b_h, trace=False):
    nc = build()
    in_maps = make_in_maps(x, h_prev, W_z, b_z, W_h, b_h)
    res = run_bass_kernel_spmd(
        nc, in_maps, core_ids=list(range(NCORES)), trace=trace
    )
    out = np.concatenate([r["out"] for r in res.results], axis=0)
    if trace:
        _CACHE["last_results"] = res
    return out


# revision 27
# speedup vs baseline: 1.0793x; 1.0793x over previous
"""MinGRU cell on 8 Trainium2 NeuronCores (Bass/Tile).

Math (per batch b, hidden h):
    gz = x @ W_z^T ; gh = x @ W_h^T                 (two GEMMs, K=D=1024)
    z  = sigmoid(gz + b_z)
    h_t = (1 - z_t) * h_{t-1} + z_t * (gh_t + b_h)  (affine scan over T)

Distribution: data-parallel over batch B=16 -> 2 batches per core, weights
replicated; no cross-core communication.

Per-core pipeline, software-pipelined over a step table (7 steps of 512
tokens + 2 final steps of 256 so the output drain of the last tokens
overlaps compute; 256-col matmuls still hide their LDWEIGHTS):
  x [t,d] --SWDGE cast-load--> bf16 --PE transpose--> xT [d,t]
  GEMMs with W^T stationary (bf16, fp32 PSUM accumulation), output [h, t]
  ACT: a = sigmoid(-gz - b_z) = 1-z ; z = sigmoid(gz + b_z)
  DVE: bsc = (gh + b_h) * z ; h = tensor_tensor_scan(a, bsc) along t
  PE transpose h back to [t, h], copy to fp32 (ACT/DVE split), DMA out
  per h-half.

Steady state runs the PE at ~99% of the bf16 roofline (GEMM matmuls issue
every 216ns, transposes every 56ns, stationary loads hidden). Prologue:
x step-0 loads first (they gate the first GEMM), W^T built in bf16 from
per-128-row SWDGE chunk loads, W transposes emitted behind the step-0
GEMM pairs, junk-matmul warmup on a memset tile from ~4.7us so the HAM
clock gate opens once and stays open.
"""

import sys

sys.path.insert(0, "/opt/trn_rl_repo")

from contextlib import ExitStack

import numpy as np

import concourse.bass as bass
import concourse.mybir as mybir
import concourse.tile as tile
from concourse import bacc
from concourse.bass import ts, ds
from concourse.bass_utils import run_bass_kernel_spmd
from concourse.masks import make_identity

B, T, D, H = 16, 2048, 1024, 1024
NCORES = 8
B_LOC = B // NCORES  # 2
P = 128
TC = 512  # max tokens per step
TSUB = TC // P  # 4
DC = D // P  # 8 contraction chunks
HC = H // P  # 8 hidden chunks
HH = H // 2
NWARM = 75  # junk matmuls to open the PE clock gate during the DMA prologue

# Step table: (batch, start token, length). The last 512 tokens run as two
# 256-token steps to halve the final output drain.
STEPS = (
    [(0, t0, TC) for t0 in range(0, T, TC)]
    + [(1, t0, TC) for t0 in range(0, T - TC, TC)]
    + [(1, T - TC, TC // 2), (1, T - TC // 2, TC // 2)]
)
NSTEP = len(STEPS)  # 9

F32 = mybir.dt.float32
BF16 = mybir.dt.bfloat16
AF = mybir.ActivationFunctionType
OP = mybir.AluOpType

_CACHE = {}


class _State:
    pass


def _mingru_tile(tc, out, x, h0, wz, bz, wh, bh):
    nc = tc.nc
    st = _State()

    with ExitStack() as ctx:
        consts = ctx.enter_context(tc.tile_pool(name="consts", bufs=1))

        # Junk-warmup operand: a memset tile is ready ~4us before the
        # iota-built identity, letting the PE clock ramp start earlier.
        junk_mv = consts.tile([P, P], BF16, name="junk_mv")
        nc.gpsimd.memset(junk_mv, 0)

        # --- SBUF pools --------------------------------------------------
        wn_p = ctx.enter_context(tc.tile_pool(name="wn", bufs=1))
        xn_p = ctx.enter_context(tc.tile_pool(name="xn", bufs=2))
        xt_p = ctx.enter_context(tc.tile_pool(name="xt", bufs=2))
        azb_p = ctx.enter_context(tc.tile_pool(name="azb", bufs=2))
        scan_p = ctx.enter_context(tc.tile_pool(name="scan", bufs=2))
        on_p = ctx.enter_context(tc.tile_pool(name="on", bufs=3))

        st.xn = {}  # step -> [j natural x tiles [P, D] bf16]
        st.xt = {}  # step -> [4 xT pair tiles [P, 2*tl] bf16] (dc pair p)
        st.scan = {}  # step -> [8 scan tiles [P, tl] bf16]
        st.wt = {"z": [None] * HC, "h": [None] * HC}
        st.wn = {"z": [None] * HC, "h": [None] * HC}

        def sect_A(s):  # x cast-loads (SWDGE)
            b, t0, tl = STEPS[s]
            tiles = []
            for j in range(tl // P):
                xt_nat = xn_p.tile([P, D], BF16, tag=f"xn{j}", name=f"xn_{s}_{j}")
                nc.gpsimd.dma_start(
                    out=xt_nat, in_=x[b, ds(t0 + j * P, P), :]
                )
                tiles.append(xt_nat)
            st.xn[s] = tiles

        def load_w_chunk(hc):  # SWDGE cast-load one [128h, D] chunk of each W
            for w_ap, wi in ((wz, "z"), (wh, "h")):
                t_ = wn_p.tile([P, D], BF16, tag=f"wn_{wi}_{hc}",
                               name=f"wn_{wi}_{hc}")
                nc.gpsimd.dma_start(out=t_, in_=w_ap[ts(hc, P), :])
                st.wn[wi][hc] = t_

        def build_wt(wi, hc):  # PE-transpose one W chunk (bf16), copy to SBUF
            pw = pxt_p.tile([P, H], BF16, tag="pxt", name=f"pw_{wi}_{hc}")
            wnt = st.wn[wi][hc]
            for dc in range(DC):
                nc.tensor.transpose(pw[:, ts(dc, P)], wnt[:, ts(dc, P)], id_bf)
            wt_sb = consts.tile([P, D], BF16, name=f"wt_{wi}_{hc}")
            # W_z copies on ACT, W_h copies on DVE (keeps both queues clear
            # of cross-waits against the step-0 GEMM stream).
            if wi == "z":
                nc.scalar.copy(wt_sb, pw[:, :D])
            else:
                nc.vector.tensor_copy(wt_sb, pw[:, :D])
            st.wt[wi][hc] = wt_sb

        def sect_B(s):  # x transposes (PE) into [P, 2*tl] pair tiles
            b, t0, tl = STEPS[s]
            jn = tl // P
            xn = st.xn.pop(s)
            tiles = []
            for p_ in range(DC // 2):
                pxt = pxt_p.tile([P, H], BF16, tag="pxt", name=f"pxt_{s}_{p_}")
                for q in range(2):
                    for j in range(jn):
                        nc.tensor.transpose(
                            pxt[:, ds(q * tl + j * P, P)],
                            xn[j][:, ts(2 * p_ + q, P)],
                            id_bf,
                        )
                xt_sb = xt_p.tile([P, 2 * TC], BF16, tag=f"xt{p_}",
                                  name=f"xt_{s}_{p_}")
                # split psum->SBUF copies across ACT / DVE
                if p_ < 2:
                    nc.scalar.copy(xt_sb[:, : 2 * tl], pxt[:, : 2 * tl])
                else:
                    nc.vector.tensor_copy(xt_sb[:, : 2 * tl], pxt[:, : 2 * tl])
                tiles.append(xt_sb)
            st.xt[s] = tiles

        def gemm(s, hc, which):
            b, t0, tl = STEPS[s]
            xts = st.xt[s]
            wt = st.wt[which][hc]
            pool = pz_p if which == "z" else ph_p
            psum = pool.tile(
                [P, TC], F32, tag="pz" if which == "z" else "ph",
                name=f"ps{which}_{s}_{hc}",
            )
            for dc in range(DC):
                nc.tensor.matmul(
                    psum[:, :tl],
                    wt[:, ts(dc, P)],
                    xts[dc // 2][:, ds((dc % 2) * tl, tl)],
                    start=(dc == 0),
                    stop=(dc == DC - 1),
                )
            return psum

        def sect_post(s, hc, psum_z, psum_h):
            b, t0, tl = STEPS[s]
            a_sb = azb_p.tile([P, TC], F32, tag="a", name=f"a_{s}_{hc}")
            nc.scalar.activation(
                a_sb[:, :tl], psum_z[:, :tl], AF.Sigmoid,
                bias=nbz_sb[:, hc : hc + 1], scale=-1.0,
            )
            z_sb = azb_p.tile([P, TC], F32, tag="z", name=f"z_{s}_{hc}")
            nc.scalar.activation(
                z_sb[:, :tl], psum_z[:, :tl], AF.Sigmoid,
                bias=bz_sb[:, hc : hc + 1], scale=1.0,
            )
            bsc = azb_p.tile([P, TC], F32, tag="b", name=f"b_{s}_{hc}")
            nc.vector.scalar_tensor_tensor(
                bsc[:, :tl], psum_h[:, :tl], bh_sb[:, hc : hc + 1],
                z_sb[:, :tl], op0=OP.add, op1=OP.mult,
            )
            # bf16 scan output: the scan's accumulator state is fp32 in HW
            # regardless of out dtype, so only stored values round (~2^-9).
            sc = scan_p.tile([P, TC], BF16, tag=f"sc{hc}", name=f"sc_{s}_{hc}")
            if t0 == 0:
                init = hp_sb[:, b * HC + hc : b * HC + hc + 1]
            else:
                ptl = STEPS[s - 1][2]
                init = st.scan[s - 1][hc][:, ptl - 1 : ptl]
            nc.vector.tensor_tensor_scan(
                sc[:, :tl], a_sb[:, :tl], bsc[:, :tl], init,
                op0=OP.mult, op1=OP.add,
            )
            st.scan.setdefault(s, [None] * HC)[hc] = sc

        def sect_E(s):  # out transposes (PE), copy to f32, store per h-half
            b, t0, tl = STEPS[s]
            scans = st.scan[s]
            for j in range(tl // P):
                po = pxt_p.tile([P, H], BF16, tag="pxt", name=f"po_{s}_{j}")
                for hc in range(HC):
                    nc.tensor.transpose(
                        po[:, ts(hc, P)], scans[hc][:, ts(j, P)], id_bf
                    )
                for hh in range(2):
                    on = on_p.tile([P, HH], F32, tag=f"on{hh}",
                                   name=f"on_{s}_{j}_{hh}")
                    # split the psum->fp32 casts across ACT / DVE
                    if hh == 0:
                        nc.scalar.copy(on, po[:, ds(0, HH)])
                    else:
                        nc.vector.tensor_copy(on, po[:, ds(HH, HH)])
                    nc.sync.dma_start(
                        out=out[b, ds(t0 + j * P, P), ds(hh * HH, HH)],
                        in_=on,
                    )
            if s - 1 in st.scan:
                del st.scan[s - 1]

        # --- prologue ----------------------------------------------------
        # Pool (SWDGE) queue order = criticality: x step 0 first (it gates
        # the first GEMM via the x transposes), then identity (gpsimd iota,
        # before the W chunk generations), then the W chunks.
        sect_A(0)

        id_bf = consts.tile([P, P], BF16)
        make_identity(nc, id_bf)

        for hc in range(HC):
            load_w_chunk(hc)

        bz_sb = consts.tile([P, HC], F32)
        nc.sync.dma_start(out=bz_sb, in_=bz.rearrange("(c p) -> p c", p=P))
        bh_sb = consts.tile([P, HC], F32)
        nc.sync.dma_start(out=bh_sb, in_=bh.rearrange("(c p) -> p c", p=P))
        nbz_sb = consts.tile([P, HC], F32)
        nc.vector.tensor_scalar_mul(nbz_sb, bz_sb, -1.0)
        hp_sb = consts.tile([P, B_LOC * HC], F32)
        nc.sync.dma_start(out=hp_sb, in_=h0.rearrange("b (c p) -> p (b c)", p=P))

        # HAM warmup: back-to-back junk matmuls so the PE clock gate opens
        # while the prologue DMAs stream. The warm PSUM pool must close
        # before pz/ph/pxt open (8-bank budget).
        with tc.tile_pool(name="warm", bufs=1, space="PSUM") as warm_p, \
             tc.tile_pool(name="wdram", bufs=1, space="DRAM") as wdram_p:
            junk_ps = warm_p.tile([P, P], F32, name="junk_ps")
            for i in range(NWARM):
                nc.tensor.matmul(
                    junk_ps, junk_mv, junk_mv, start=(i == 0), stop=(i == NWARM - 1)
                )
            junk_sb = consts.tile([P, P], F32, name="junk_sb")
            nc.vector.tensor_copy(junk_sb, junk_ps)
            junk_dr = wdram_p.tile([P, P], F32, name="junk_dr")
            nc.sync.dma_start(out=junk_dr, in_=junk_sb)

        # PSUM: pz(3) + ph(3) + pxt(2) = 8 banks
        pz_p = ctx.enter_context(tc.tile_pool(name="pz", bufs=3, space="PSUM"))
        ph_p = ctx.enter_context(tc.tile_pool(name="ph", bufs=3, space="PSUM"))
        pxt_p = ctx.enter_context(tc.tile_pool(name="pxt", bufs=2, space="PSUM"))

        sect_B(0)
        # W^T chunks 0-1 ahead of the GEMM stream (2-chunk lookahead; the
        # remaining chunks are emitted behind each step-0 GEMM pair).
        for hc in range(2):
            build_wt("z", hc)
            build_wt("h", hc)

        # --- steady state ------------------------------------------------
        for s in range(NSTEP):
            if s + 1 < NSTEP:
                sect_A(s + 1)
            for hc in range(HC):
                psum_z = gemm(s, hc, "z")
                psum_h = gemm(s, hc, "h")
                sect_post(s, hc, psum_z, psum_h)
                if s == 0 and hc + 2 < HC:
                    build_wt("z", hc + 2)
                    build_wt("h", hc + 2)
                if s >= 1 and hc == 2:
                    sect_E(s - 1)
                if s + 1 < NSTEP and hc == 4:
                    sect_B(s + 1)
        sect_E(NSTEP - 1)


def build():
    if "nc" in _CACHE:
        return _CACHE["nc"]
    nc = bacc.Bacc(
        "TRN2", target_bir_lowering=False, debug=False, num_devices=NCORES
    )
    x = nc.dram_tensor("x", [B_LOC, T, D], F32, kind="ExternalInput").ap()
    h0 = nc.dram_tensor("h0", [B_LOC, H], F32, kind="ExternalInput").ap()
    wz = nc.dram_tensor("wz", [H, D], F32, kind="ExternalInput").ap()
    bz = nc.dram_tensor("bz", [H], F32, kind="ExternalInput").ap()
    wh = nc.dram_tensor("wh", [H, D], F32, kind="ExternalInput").ap()
    bh = nc.dram_tensor("bh", [H], F32, kind="ExternalInput").ap()
    out = nc.dram_tensor("out", [B_LOC, T, H], F32, kind="ExternalOutput").ap()
    with tile.TileContext(nc) as tctx:
        _mingru_tile(tctx, out, x, h0, wz, bz, wh, bh)
    nc.compile()
    _CACHE["nc"] = nc
    return nc


def make_in_maps(x, h_prev, W_z, b_z, W_h, b_h):
    x = np.ascontiguousarray(np.asarray(x, dtype=np.float32))
    h_prev = np.ascontiguousarray(np.asarray(h_prev, dtype=np.float32))
    W_z = np.ascontiguousarray(np.asarray(W_z, dtype=np.float32))
    b_z = np.ascontiguousarray(np.asarray(b_z, dtype=np.float32))
    W_h = np.ascontiguousarray(np.asarray(W_h, dtype=np.float32))
    b_h = np.ascontiguousarray(np.asarray(b_h, dtype=np.float32))
    in_maps = []
    for c in range(NCORES):
        sl = slice(c * B_LOC, (c + 1) * B_LOC)
        in_maps.append(
            {
                "x": x[sl],
                "h0": h_prev[sl],
                "wz": W_z,
                "bz": b_z,
                "wh": W_h,
                "bh": b_h,
            }
        )
    return in_maps


def kernel(x, h_prev, W_z, b_z, W_h, b_h, trace=False):
    nc = build()
    in_maps = make_in_maps(x, h_prev, W_z, b_z, W_h, b_h)
    res = run_bass_kernel_spmd(
        nc, in_maps, core_ids=list(range(NCORES)), trace=trace
    )
    out = np.concatenate([r["out"] for r in res.results], axis=0)
    if trace:
        _CACHE["last_results"] = res
    return out


# revision 30
# speedup vs baseline: 1.1094x; 1.0278x over previous
"""MinGRU cell on 8 Trainium2 NeuronCores (Bass/Tile).

Math (per batch b, hidden h):
    gz = x @ W_z^T ; gh = x @ W_h^T                 (two GEMMs, K=D=1024)
    z  = sigmoid(gz + b_z)
    h_t = (1 - z_t) * h_{t-1} + z_t * (gh_t + b_h)  (affine scan over T)

Distribution: data-parallel over batch B=16 -> 2 batches per core, weights
replicated; no cross-core communication.

Per-core pipeline, software-pipelined over a step table (7 steps of 512
tokens + 2 final steps of 256 so the output drain of the last tokens
overlaps compute; 256-col matmuls still hide their LDWEIGHTS):
  x [t,d] --SWDGE cast-load--> bf16 --PE transpose--> xT [d,t]
  GEMMs with W^T stationary (bf16, fp32 PSUM accumulation), output [h, t]
  ACT: a = sigmoid(-gz - b_z) = 1-z ; z = sigmoid(gz + b_z)
  DVE: bsc = (gh + b_h) * z ; h = tensor_tensor_scan(a, bsc) along t
  PE transpose h back to [t, h], copy to fp32 (ACT/DVE split), DMA out
  per h-half.

Steady state runs the PE at ~99% of the bf16 roofline (GEMM matmuls issue
every 216ns, transposes every 56ns, stationary loads hidden). Prologue:
x step-0 loads first (they gate the first GEMM), W^T built in bf16 from
per-128-row SWDGE chunk loads, W transposes emitted behind the step-0
GEMM pairs, junk-matmul warmup on a memset tile from ~4.7us so the HAM
clock gate opens once and stays open.
"""

import sys

sys.path.insert(0, "/opt/trn_rl_repo")

from contextlib import ExitStack

import numpy as np

import concourse.bass as bass
import concourse.mybir as mybir
import concourse.tile as tile
from concourse import bacc
from concourse.bass import ts, ds
from concourse.bass_utils import run_bass_kernel_spmd
from concourse.masks import make_identity

B, T, D, H = 16, 2048, 1024, 1024
NCORES = 8
B_LOC = B // NCORES  # 2
P = 128
TC = 512  # max tokens per step
TSUB = TC // P  # 4
DC = D // P  # 8 contraction chunks
HC = H // P  # 8 hidden chunks
HH = H // 2
NWARM = 100  # junk matmuls to open the PE clock gate during the DMA prologue

# Step table: (batch, start token, length). The last 512 tokens run as two
# 256-token steps to halve the final output drain.
STEPS = (
    [(0, t0, TC) for t0 in range(0, T, TC)]
    + [(1, t0, TC) for t0 in range(0, T - TC, TC)]
    + [(1, T - TC, TC // 2), (1, T - TC // 2, TC // 2)]
)
NSTEP = len(STEPS)  # 9

F32 = mybir.dt.float32
BF16 = mybir.dt.bfloat16
AF = mybir.ActivationFunctionType
OP = mybir.AluOpType

_CACHE = {}


class _State:
    pass


def _mingru_tile(tc, out, x, h0, wz, bz, wh, bh):
    nc = tc.nc
    st = _State()

    with ExitStack() as ctx:
        consts = ctx.enter_context(tc.tile_pool(name="consts", bufs=1))

        # Junk-warmup operand: a memset tile is ready ~4us before the
        # iota-built identity, letting the PE clock ramp start earlier.
        junk_mv = consts.tile([P, P], BF16, name="junk_mv")
        nc.gpsimd.memset(junk_mv, 0)

        # --- SBUF pools --------------------------------------------------
        wn_p = ctx.enter_context(tc.tile_pool(name="wn", bufs=1))
        xn_p = ctx.enter_context(tc.tile_pool(name="xn", bufs=2))
        xt_p = ctx.enter_context(tc.tile_pool(name="xt", bufs=2))
        azb_p = ctx.enter_context(tc.tile_pool(name="azb", bufs=2))
        scan_p = ctx.enter_context(tc.tile_pool(name="scan", bufs=2))
        on_p = ctx.enter_context(tc.tile_pool(name="on", bufs=3))

        st.xn = {}  # step -> [j natural x tiles [P, D] bf16]
        st.xt = {}  # step -> [4 xT pair tiles [P, 2*tl] bf16] (dc pair p)
        st.scan = {}  # step -> [8 scan tiles [P, tl] bf16]
        st.wt = {"z": [None] * HC, "h": [None] * HC}
        st.wn = {"z": [None] * HC, "h": [None] * HC}

        def sect_A(s):  # x cast-loads (SWDGE)
            b, t0, tl = STEPS[s]
            tiles = []
            for j in range(tl // P):
                xt_nat = xn_p.tile([P, D], BF16, tag=f"xn{j}", name=f"xn_{s}_{j}")
                nc.gpsimd.dma_start(
                    out=xt_nat, in_=x[b, ds(t0 + j * P, P), :]
                )
                tiles.append(xt_nat)
            st.xn[s] = tiles

        def load_w_chunk(hc):  # SWDGE cast-load one [128h, D] chunk of each W
            for w_ap, wi in ((wz, "z"), (wh, "h")):
                t_ = wn_p.tile([P, D], BF16, tag=f"wn_{wi}_{hc}",
                               name=f"wn_{wi}_{hc}")
                nc.gpsimd.dma_start(out=t_, in_=w_ap[ts(hc, P), :])
                st.wn[wi][hc] = t_

        def build_wt(wi, hc):  # PE-transpose one W chunk (bf16), copy to SBUF
            pw = pxt_p.tile([P, H], BF16, tag="pxt", name=f"pw_{wi}_{hc}")
            wnt = st.wn[wi][hc]
            for dc in range(DC):
                nc.tensor.transpose(pw[:, ts(dc, P)], wnt[:, ts(dc, P)], id_bf)
            wt_sb = consts.tile([P, D], BF16, name=f"wt_{wi}_{hc}")
            # W_z copies on ACT, W_h copies on DVE (keeps both queues clear
            # of cross-waits against the step-0 GEMM stream).
            if wi == "z":
                nc.scalar.copy(wt_sb, pw[:, :D])
            else:
                nc.vector.tensor_copy(wt_sb, pw[:, :D])
            st.wt[wi][hc] = wt_sb

        def sect_B(s):  # x transposes (PE) into [P, 2*tl] pair tiles
            b, t0, tl = STEPS[s]
            jn = tl // P
            xn = st.xn.pop(s)
            tiles = []
            for p_ in range(DC // 2):
                pxt = pxt_p.tile([P, H], BF16, tag="pxt", name=f"pxt_{s}_{p_}")
                for q in range(2):
                    for j in range(jn):
                        nc.tensor.transpose(
                            pxt[:, ds(q * tl + j * P, P)],
                            xn[j][:, ts(2 * p_ + q, P)],
                            id_bf,
                        )
                xt_sb = xt_p.tile([P, 2 * TC], BF16, tag=f"xt{p_}",
                                  name=f"xt_{s}_{p_}")
                # split psum->SBUF copies across ACT / DVE
                if p_ < 2:
                    nc.scalar.copy(xt_sb[:, : 2 * tl], pxt[:, : 2 * tl])
                else:
                    nc.vector.tensor_copy(xt_sb[:, : 2 * tl], pxt[:, : 2 * tl])
                tiles.append(xt_sb)
            st.xt[s] = tiles

        def gemm(s, hc, which):
            b, t0, tl = STEPS[s]
            xts = st.xt[s]
            wt = st.wt[which][hc]
            pool = pz_p if which == "z" else ph_p
            psum = pool.tile(
                [P, TC], F32, tag="pz" if which == "z" else "ph",
                name=f"ps{which}_{s}_{hc}",
            )
            for dc in range(DC):
                nc.tensor.matmul(
                    psum[:, :tl],
                    wt[:, ts(dc, P)],
                    xts[dc // 2][:, ds((dc % 2) * tl, tl)],
                    start=(dc == 0),
                    stop=(dc == DC - 1),
                )
            return psum

        def sect_post(s, hc, psum_z, psum_h):
            b, t0, tl = STEPS[s]
            a_sb = azb_p.tile([P, TC], F32, tag="a", name=f"a_{s}_{hc}")
            nc.scalar.activation(
                a_sb[:, :tl], psum_z[:, :tl], AF.Sigmoid,
                bias=nbz_sb[:, hc : hc + 1], scale=-1.0,
            )
            z_sb = azb_p.tile([P, TC], F32, tag="z", name=f"z_{s}_{hc}")
            nc.scalar.activation(
                z_sb[:, :tl], psum_z[:, :tl], AF.Sigmoid,
                bias=bz_sb[:, hc : hc + 1], scale=1.0,
            )
            bsc = azb_p.tile([P, TC], F32, tag="b", name=f"b_{s}_{hc}")
            nc.vector.scalar_tensor_tensor(
                bsc[:, :tl], psum_h[:, :tl], bh_sb[:, hc : hc + 1],
                z_sb[:, :tl], op0=OP.add, op1=OP.mult,
            )
            # bf16 scan output: the scan's accumulator state is fp32 in HW
            # regardless of out dtype, so only stored values round (~2^-9).
            sc = scan_p.tile([P, TC], BF16, tag=f"sc{hc}", name=f"sc_{s}_{hc}")
            if t0 == 0:
                init = hp_sb[:, b * HC + hc : b * HC + hc + 1]
            else:
                ptl = STEPS[s - 1][2]
                init = st.scan[s - 1][hc][:, ptl - 1 : ptl]
            nc.vector.tensor_tensor_scan(
                sc[:, :tl], a_sb[:, :tl], bsc[:, :tl], init,
                op0=OP.mult, op1=OP.add,
            )
            st.scan.setdefault(s, [None] * HC)[hc] = sc

        def sect_E(s):  # out transposes (PE), copy to f32, store per h-half
            b, t0, tl = STEPS[s]
            scans = st.scan[s]
            for j in range(tl // P):
                po = pxt_p.tile([P, H], BF16, tag="pxt", name=f"po_{s}_{j}")
                for hc in range(HC):
                    nc.tensor.transpose(
                        po[:, ts(hc, P)], scans[hc][:, ts(j, P)], id_bf
                    )
                for hh in range(2):
                    on = on_p.tile([P, HH], F32, tag=f"on{hh}",
                                   name=f"on_{s}_{j}_{hh}")
                    # split the psum->fp32 casts across ACT / DVE
                    if hh == 0:
                        nc.scalar.copy(on, po[:, ds(0, HH)])
                    else:
                        nc.vector.tensor_copy(on, po[:, ds(HH, HH)])
                    nc.sync.dma_start(
                        out=out[b, ds(t0 + j * P, P), ds(hh * HH, HH)],
                        in_=on,
                    )
            if s - 1 in st.scan:
                del st.scan[s - 1]

        # --- prologue ----------------------------------------------------
        # Pool (SWDGE) queue order = criticality: x step 0 first (it gates
        # the first GEMM via the x transposes), then identity (gpsimd iota,
        # before the W chunk generations), then the W chunks.
        sect_A(0)

        id_bf = consts.tile([P, P], BF16)
        make_identity(nc, id_bf)

        # Interleave step-1's x loads among the W chunks: x1 queued after all
        # W starves step 1 at the boundary (~5us PE stall), x1 fully before W
        # starves the step-0 W^T builds instead. One x tile per W pair keeps
        # both streams ahead of their consumers.
        b1, t1, _ = STEPS[1]
        st.xn[1] = []
        for hc in range(HC):
            load_w_chunk(hc)
            if 1 <= hc <= 4:
                j = hc - 1
                t_ = xn_p.tile([P, D], BF16, tag=f"xn{j}", name=f"xn_1_{j}")
                nc.gpsimd.dma_start(out=t_, in_=x[b1, ds(t1 + j * P, P), :])
                st.xn[1].append(t_)

        bz_sb = consts.tile([P, HC], F32)
        nc.sync.dma_start(out=bz_sb, in_=bz.rearrange("(c p) -> p c", p=P))
        bh_sb = consts.tile([P, HC], F32)
        nc.sync.dma_start(out=bh_sb, in_=bh.rearrange("(c p) -> p c", p=P))
        nbz_sb = consts.tile([P, HC], F32)
        nc.vector.tensor_scalar_mul(nbz_sb, bz_sb, -1.0)
        hp_sb = consts.tile([P, B_LOC * HC], F32)
        nc.sync.dma_start(out=hp_sb, in_=h0.rearrange("b (c p) -> p (b c)", p=P))

        # HAM warmup: back-to-back junk matmuls so the PE clock gate opens
        # while the prologue DMAs stream. The warm PSUM pool must close
        # before pz/ph/pxt open (8-bank budget).
        with tc.tile_pool(name="warm", bufs=1, space="PSUM") as warm_p, \
             tc.tile_pool(name="wdram", bufs=1, space="DRAM") as wdram_p:
            junk_ps = warm_p.tile([P, P], F32, name="junk_ps")
            for i in range(NWARM):
                nc.tensor.matmul(
                    junk_ps, junk_mv, junk_mv, start=(i == 0), stop=(i == NWARM - 1)
                )
            junk_sb = consts.tile([P, P], F32, name="junk_sb")
            nc.vector.tensor_copy(junk_sb, junk_ps)
            junk_dr = wdram_p.tile([P, P], F32, name="junk_dr")
            nc.sync.dma_start(out=junk_dr, in_=junk_sb)

        # PSUM: pz(3) + ph(3) + pxt(2) = 8 banks
        pz_p = ctx.enter_context(tc.tile_pool(name="pz", bufs=3, space="PSUM"))
        ph_p = ctx.enter_context(tc.tile_pool(name="ph", bufs=3, space="PSUM"))
        pxt_p = ctx.enter_context(tc.tile_pool(name="pxt", bufs=2, space="PSUM"))

        sect_B(0)
        # W^T chunks 0-1 ahead of the GEMM stream (2-chunk lookahead; the
        # remaining chunks are emitted behind each step-0 GEMM pair).
        for hc in range(2):
            build_wt("z", hc)
            build_wt("h", hc)

        # --- steady state ------------------------------------------------
        for s in range(NSTEP):
            if 1 <= s and s + 1 < NSTEP:
                sect_A(s + 1)
            for hc in range(HC):
                psum_z = gemm(s, hc, "z")
                psum_h = gemm(s, hc, "h")
                sect_post(s, hc, psum_z, psum_h)
                if s == 0 and hc + 2 < HC:
                    build_wt("z", hc + 2)
                    build_wt("h", hc + 2)
                if s >= 1 and hc == 2:
                    sect_E(s - 1)
                # step 0's x-transposes one pair later: x1 streams in behind
                # the W chunks and needs the extra slack.
                if s + 1 < NSTEP and hc == (5 if s == 0 else 4):
                    sect_B(s + 1)
        sect_E(NSTEP - 1)


def build():
    if "nc" in _CACHE:
        return _CACHE["nc"]
    nc = bacc.Bacc(
        "TRN2", target_bir_lowering=False, debug=False, num_devices=NCORES
    )
    x = nc.dram_tensor("x", [B_LOC, T, D], F32, kind="ExternalInput").ap()
    h0 = nc.dram_tensor("h0", [B_LOC, H], F32, kind="ExternalInput").ap()
    wz = nc.dram_tensor("wz", [H, D], F32, kind="ExternalInput").ap()
    bz = nc.dram_tensor("bz", [H], F32, kind="ExternalInput").ap()
    wh = nc.dram_tensor("wh", [H, D], F32, kind="ExternalInput").ap()
    bh = nc.dram_tensor("bh", [H], F32, kind="ExternalInput").ap()
    out = nc.dram_tensor("out", [B_LOC, T, H], F32, kind="ExternalOutput").ap()
    with tile.TileContext(nc) as tctx:
        _mingru_tile(tctx, out, x, h0, wz, bz, wh, bh)
    nc.compile()
    _CACHE["nc"] = nc
    return nc


def make_in_maps(x, h_prev, W_z, b_z, W_h, b_h):
    x = np.ascontiguousarray(np.asarray(x, dtype=np.float32))
    h_prev = np.ascontiguousarray(np.asarray(h_prev, dtype=np.float32))
    W_z = np.ascontiguousarray(np.asarray(W_z, dtype=np.float32))
    b_z = np.ascontiguousarray(np.asarray(b_z, dtype=np.float32))
    W_h = np.ascontiguousarray(np.asarray(W_h, dtype=np.float32))
    b_h = np.ascontiguousarray(np.asarray(b_h, dtype=np.float32))
    in_maps = []
    for c in range(NCORES):
        sl = slice(c * B_LOC, (c + 1) * B_LOC)
        in_maps.append(
            {
                "x": x[sl],
                "h0": h_prev[sl],
                "wz": W_z,
                "bz": b_z,
                "wh": W_h,
                "bh": b_h,
            }
        )
    return in_maps


def kernel(x, h_prev, W_z, b_z, W_h, b_h, trace=False):
    nc = build()
    in_maps = make_in_maps(x, h_prev, W_z, b_z, W_h, b_h)
    res = run_bass_kernel_spmd(
        nc, in_maps, core_ids=list(range(NCORES)), trace=trace
    )
    out = np.concatenate([r["out"] for r in res.results], axis=0)
    if trace:
        _CACHE["last_results"] = res
    return out
